# revision 7
# baseline (speedup 1.0000x reference)
"""GraphSAGE (2-layer, mean-agg) Trainium2 Bass kernel, 8-core SPMD.

Design: shard dst nodes across 8 cores (6250 each). Edges partitioned by dst
owner, sorted by dst, grouped into 128-dst windows. Messages fetched with
gpsimd dma_gather (bf16 tables, single_packet=False); segment-sum done on the
PE via per-rank selection-matrix matmuls accumulated in PSUM per window
(scatter-add CCE has a duplicate-index race on HW, so no scatters are used).
Layer-2 aggregates p = h @ w2_l (40->128-col padded bf16) instead of h
(512-dim): p shards are AllGathered in two 3125-row slices so gather indices
fit int16.
"""
import numpy as np
import ml_dtypes

N = 50000
E = 800000
DIN, HID, OUT = 128, 512, 40
NCORES = 8
NLOC = N // NCORES          # 6250
P = 128
NWIN = (NLOC + P - 1) // P  # 49
NPAD = NWIN * P             # 6272
XSPLIT = 32768              # x table split for int16 gather indices
PSLICE = NLOC // 2          # 3125: p-table allgather slice
L1_CHUNK = 2                # windows per L1 gather call group
L2_CHUNK = 4

bf16 = ml_dtypes.bfloat16


def _build_schedule(edge_index):
    """Per-core, per-layer edge orderings + the cross-core-common rank schedule."""
    src = np.asarray(edge_index[0], dtype=np.int64)
    dst = np.asarray(edge_index[1], dtype=np.int64)
    deg = np.bincount(dst, minlength=N).astype(np.float32)
    recip = 1.0 / np.maximum(deg, 1.0)

    per_core = []
    for c in range(NCORES):
        lo, hi = c * NLOC, (c + 1) * NLOC
        m = (dst >= lo) & (dst < hi)
        s, d = src[m], dst[m] - lo
        per_core.append((s, d))

    # bucket key per layer: L1 by src>=XSPLIT, L2 by (src % NLOC) // PSLICE
    def buckets(s):
        return [s >= XSPLIT, (s % NLOC) // PSLICE]

    # counts[layer][core][win][bucket]
    counts = np.zeros((2, NCORES, NWIN, 2), np.int64)
    percore_lists = []  # [core][layer][win][bucket] -> (gidx array, drel array)
    for c in range(NCORES):
        s, d = per_core[c]
        w = d // P
        bk = buckets(s)
        layers = []
        for L in range(2):
            b = bk[L].astype(np.int64)
            order = np.lexsort((b, w))  # by window, then bucket
            ss, dd, ww, bb = s[order], d[order], w[order], b[order]
            wins = []
            for wi in range(NWIN):
                sel = ww == wi
                ssw, ddw, bbw = ss[sel], dd[sel], bb[sel]
                ents = []
                for bu in range(2):
                    q = bbw == bu
                    sq, dq = ssw[q], ddw[q]
                    if L == 0:
                        gi = np.where(sq >= XSPLIT, sq - XSPLIT, sq)
                    else:
                        u = sq % NLOC
                        gi = (sq // NLOC) * PSLICE + (u % PSLICE)
                    counts[L, c, wi, bu] = len(sq)
                    ents.append((gi.astype(np.int64), (dq - wi * P).astype(np.int64)))
                wins.append(ents)
            layers.append(wins)
        percore_lists.append(layers)

    # common rank schedule: ranks[L][win][bucket] = ceil(max_c count /128), >=0
    ranks = np.zeros((2, NWIN, 2), np.int64)
    for L in range(2):
        mx = counts[L].max(axis=0)  # [NWIN, 2]
        ranks[L] = (mx + P - 1) // P
        for wi in range(NWIN):  # ensure every window has >=1 rank total
            if ranks[L, wi].sum() == 0:
                ranks[L, wi, 0] = 1
    return per_core, percore_lists, ranks, recip


def _wrap_call(flat_idx):
    """int16 wrapped layout for one gather call: slot i -> [i%16, i//16]."""
    n = len(flat_idx)
    w = flat_idx.astype(np.int16).reshape(n // 16, 16).T.copy()
    return np.tile(w, (8, 1))  # [128, n/16]


def _pack_layer(layers_for_core, ranks, L, chunk):
    """Build idx [128, T16] int16, drel [128, R] bf16 and call/window metadata.
    Call layout per chunk: [bucket0: win a..b segs][bucket1: win a..b segs]."""
    idx_cols, drel_cols = [], []
    calls = []      # (bucket, rank_off, n_ranks)
    win_ranges = [] # per window: list of (rank_start, rank_end)
    rank_off = 0
    for c0 in range(0, NWIN, chunk):
        cw = range(c0, min(c0 + chunk, NWIN))
        for bu in range(2):
            seg_ranks = int(sum(ranks[L, wi, bu] for wi in cw))
            if seg_ranks == 0:
                continue
            flat = np.zeros(seg_ranks * P, np.int64)
            drel = np.full(seg_ranks * P, -1, np.int64)
            off = 0
            for wi in cw:
                nr = int(ranks[L, wi, bu])
                if nr == 0:
                    continue
                gi, dq = layers_for_core[wi][bu]
                flat[off:off + len(gi)] = gi
                drel[off:off + len(dq)] = dq
                if len(win_ranges) <= wi - 0:
                    pass
                win_ranges.append((wi, rank_off + off // P, rank_off + off // P + nr))
                off += nr * P
            idx_cols.append(_wrap_call(flat))
            # drel slot i -> partition i%128, rank i//128
            drel_cols.append(drel.reshape(seg_ranks, P).T.astype(bf16))
            calls.append((bu, rank_off, seg_ranks))
            rank_off += seg_ranks
    idx_arr = np.concatenate(idx_cols, axis=1)
    drel_arr = np.concatenate(drel_cols, axis=1)
    # merge win_ranges into per-window lists
    wmap = [[] for _ in range(NWIN)]
    for wi, a, b in win_ranges:
        wmap[wi].append((a, b))
    return idx_arr, drel_arr, calls, wmap


def kernel(x, edge_index, w1_l, b1, w1_r, w2_l, b2, w2_r):
    import concourse.bacc as bacc
    import concourse.bass as bass
    import concourse.mybir as mybir
    import concourse.tile as tile
    from concourse.bass_utils import run_bass_kernel_spmd
    from concourse.library_config import mlp
    from concourse.masks import make_identity

    x = np.asarray(x, np.float32)
    per_core, percore_lists, ranks, recip = _build_schedule(np.asarray(edge_index))

    # ---- host-side packed arrays (same shapes on every core) ----
    core_arrays = []
    for c in range(NCORES):
        i1, d1, calls1, wmap1 = _pack_layer(percore_lists[c][0], ranks, 0, L1_CHUNK)
        i2, d2, calls2, wmap2 = _pack_layer(percore_lists[c][1], ranks, 1, L2_CHUNK)
        core_arrays.append((i1, d1, i2, d2))
    calls1, wmap1, calls2, wmap2 = calls1, wmap1, calls2, wmap2  # same all cores

    xlo = np.zeros((XSPLIT, DIN), bf16); xlo[:] = x[:XSPLIT].astype(bf16)
    xhi = np.zeros((N - XSPLIT, DIN), bf16); xhi[:] = x[XSPLIT:].astype(bf16)
    iota_np = np.tile(np.arange(P, dtype=np.float32), (P, 1)).astype(bf16)
    b2b_np = np.tile(np.asarray(b2, np.float32)[None, :], (P, 1))

    T16_1, R1 = core_arrays[0][0].shape[1], core_arrays[0][1].shape[1]
    T16_2, R2 = core_arrays[0][2].shape[1], core_arrays[0][3].shape[1]

    nc = bacc.Bacc("TRN2")
    dt = mybir.dt
    t_xlo = nc.declare_dram_parameter("xlo", [XSPLIT, DIN], dt.bfloat16, isOutput=False)
    t_xhi = nc.declare_dram_parameter("xhi", [N - XSPLIT, DIN], dt.bfloat16, isOutput=False)
    t_xoT = nc.declare_dram_parameter("xoT", [P, NPAD], dt.bfloat16, isOutput=False)
    t_i1 = nc.declare_dram_parameter("i1", [P, T16_1], dt.int16, isOutput=False)
    t_d1 = nc.declare_dram_parameter("d1", [P, R1], dt.bfloat16, isOutput=False)
    t_i2 = nc.declare_dram_parameter("i2", [P, T16_2], dt.int16, isOutput=False)
    t_d2 = nc.declare_dram_parameter("d2", [P, R2], dt.bfloat16, isOutput=False)
    t_w1l = nc.declare_dram_parameter("w1l", [DIN, HID], dt.bfloat16, isOutput=False)
    t_w1r = nc.declare_dram_parameter("w1r", [DIN, HID], dt.bfloat16, isOutput=False)
    t_w2l = nc.declare_dram_parameter("w2l", [P, HID // P, OUT], dt.bfloat16, isOutput=False)
    t_w2r = nc.declare_dram_parameter("w2r", [P, HID // P, OUT], dt.bfloat16, isOutput=False)
    t_b1 = nc.declare_dram_parameter("b1", [P, HID // P], dt.float32, isOutput=False)
    t_b2 = nc.declare_dram_parameter("b2b", [P, OUT], dt.float32, isOutput=False)
    t_rc = nc.declare_dram_parameter("rc", [P, NWIN], dt.float32, isOutput=False)
    t_iota = nc.declare_dram_parameter("iota", [P, P], dt.bfloat16, isOutput=False)
    t_out = nc.declare_dram_parameter("out", [NPAD, OUT], dt.float32, isOutput=True)

    p0 = nc.dram_tensor("p0", [PSLICE, P], dt.bfloat16)
    p1 = nc.dram_tensor("p1", [PSLICE, P], dt.bfloat16)
    pg0 = nc.dram_tensor("pg0", [NCORES * PSLICE, P], dt.bfloat16, addr_space="Shared")
    pg1 = nc.dram_tensor("pg1", [NCORES * PSLICE, P], dt.bfloat16, addr_space="Shared")

    AluOp = mybir.AluOpType
    AF = mybir.ActivationFunctionType

    with tile.TileContext(nc) as tc:
        with tc.tile_pool(name="const", bufs=1) as cpool, \
             tc.tile_pool(name="msg", bufs=2) as mpool, \
             tc.tile_pool(name="sm", bufs=3) as spool, \
             tc.tile_pool(name="work", bufs=3) as wpool, \
             tc.tile_pool(name="psumA", bufs=2, space="PSUM") as ppool, \
             tc.tile_pool(name="psumB", bufs=1, space="PSUM") as ppoolb:
            nc.gpsimd.load_library(mlp)
            ident = cpool.tile([P, P], dt.bfloat16)
            make_identity(nc, ident[:])
            iota_t = cpool.tile([P, P], dt.bfloat16)
            nc.sync.dma_start(iota_t[:], t_iota[:])
            i1_t = cpool.tile([P, T16_1], dt.int16)
            nc.sync.dma_start(i1_t[:], t_i1[:])
            d1_t = cpool.tile([P, R1], dt.bfloat16)
            nc.sync.dma_start(d1_t[:], t_d1[:])
            i2_t = cpool.tile([P, T16_2], dt.int16)
            nc.sync.dma_start(i2_t[:], t_i2[:])
            d2_t = cpool.tile([P, R2], dt.bfloat16)
            nc.sync.dma_start(d2_t[:], t_d2[:])
            xoT_t = cpool.tile([P, NPAD], dt.bfloat16)
            nc.sync.dma_start(xoT_t[:], t_xoT[:])
            w1l_t = cpool.tile([DIN, HID], dt.bfloat16)
            nc.sync.dma_start(w1l_t[:], t_w1l[:])
            w1r_t = cpool.tile([DIN, HID], dt.bfloat16)
            nc.sync.dma_start(w1r_t[:], t_w1r[:])
            w2l_t = cpool.tile([P, HID // P, OUT], dt.bfloat16)
            nc.sync.dma_start(w2l_t[:], t_w2l[:])
            w2r_t = cpool.tile([P, HID // P, OUT], dt.bfloat16)
            nc.sync.dma_start(w2r_t[:], t_w2r[:])
            b1_t = cpool.tile([P, HID // P], dt.float32)
            nc.sync.dma_start(b1_t[:], t_b1[:])
            b2_t = cpool.tile([P, OUT], dt.float32)
            nc.sync.dma_start(b2_t[:], t_b2[:])
            rc_t = cpool.tile([P, NWIN], dt.float32)
            nc.sync.dma_start(rc_t[:], t_rc[:])
            qbuf = cpool.tile([P, NWIN, OUT], dt.float32)

            # ---------- Layer 1 + stage B, chunked ----------
            call_i = 0
            cum16 = 0
            for c0 in range(0, NWIN, L1_CHUNK):
                cw = list(range(c0, min(c0 + L1_CHUNK, NWIN)))
                chunk_ranks = int(sum(ranks[0, wi, :].sum() for wi in cw))
                if chunk_ranks == 0:
                    continue
                msg = mpool.tile([P, chunk_ranks, DIN], dt.bfloat16, tag="msg1")
                base_rank = None
                # issue this chunk's gather calls
                local_off = 0
                while call_i < len(calls1):
                    bu, roff, nr = calls1[call_i]
                    # does this call belong to the current chunk?
                    if base_rank is None:
                        base_rank = roff
                    if roff - base_rank >= chunk_ranks:
                        break
                    n_idx = nr * P
                    tblap = t_xlo[:] if bu == 0 else t_xhi[:]
                    nc.gpsimd.dma_gather(
                        msg[:, roff - base_rank:roff - base_rank + nr, :],
                        tblap, i1_t[:, cum16:cum16 + n_idx // 16],
                        n_idx, n_idx, DIN, single_packet=False)
                    cum16 += n_idx // 16
                    local_off += nr
                    call_i += 1
                # per-window segmented reduction + stage B
                for wi in cw:
                    segs = [(a - base_rank, b - base_rank) for a, b in wmap1[wi]]
                    nseg = sum(b - a for a, b in segs)
                    pagg = ppool.tile([P, P], dt.float32, tag="pagg")
                    first = True
                    for a, b in segs:
                        for r in range(a, b):
                            S = spool.tile([P, P], dt.bfloat16, tag="S1")
                            nc.vector.tensor_tensor(
                                out=S[:], in0=d1_t[:, base_rank + r:base_rank + r + 1].to_broadcast([P, P]),
                                in1=iota_t[:], op=AluOp.is_equal)
                            nc.tensor.matmul(pagg[:], lhsT=S[:], rhs=msg[:, r, :],
                                             start=first, stop=(r == b - 1 and (a, b) == segs[-1]))
                            first = False
                    am = wpool.tile([P, DIN], dt.bfloat16, tag="am")
                    nc.vector.tensor_tensor(out=am[:], in0=pagg[:],
                                            in1=rc_t[:, wi:wi + 1].to_broadcast([P, DIN]),
                                            op=AluOp.mult)
                    pamT = ppoolb.tile([P, P], dt.bfloat16, tag="pamT")
                    nc.tensor.transpose(out=pamT[:], in_=am[:], identity=ident[:])
                    amT = wpool.tile([P, P], dt.bfloat16, tag="amT")
                    nc.scalar.activation(amT[:], pamT[:], AF.Copy)
                    # h blocks + p/q
                    pq = ppoolb.tile([P, OUT], dt.float32, tag="pq")
                    qq = ppoolb.tile([P, OUT], dt.float32, tag="qq")
                    for bjj in range(HID // P):
                        ph = ppool.tile([P, P], dt.float32, tag="ph")
                        nc.tensor.matmul(ph[:], lhsT=w1l_t[:, bjj * P:(bjj + 1) * P], rhs=amT[:], start=True, stop=False)
                        nc.tensor.matmul(ph[:], lhsT=w1r_t[:, bjj * P:(bjj + 1) * P], rhs=xoT_t[:, wi * P:(wi + 1) * P], start=False, stop=True)
                        hT = wpool.tile([P, P], dt.bfloat16, tag="hT")
                        nc.scalar.activation(hT[:], ph[:], AF.Relu, bias=b1_t[:, bjj:bjj + 1])
                        nc.tensor.matmul(pq[:], lhsT=hT[:], rhs=w2l_t[:, bjj, :], start=(bjj == 0), stop=(bjj == 3))
                        nc.tensor.matmul(qq[:], lhsT=hT[:], rhs=w2r_t[:, bjj, :], start=(bjj == 0), stop=(bjj == 3))
                    nc.scalar.activation(qbuf[:, wi, :], qq[:], AF.Copy)
                    pt = wpool.tile([P, P], dt.bfloat16, tag="pt")
                    nc.vector.memset(pt[:], 0.0)
                    nc.scalar.activation(pt[:, :OUT], pq[:], AF.Copy)
                    r0, r1_ = wi * P, min((wi + 1) * P, NLOC)
                    nrow = r1_ - r0
                    if r1_ <= PSLICE:
                        nc.sync.dma_start(p0[r0:r1_, :], pt[:nrow, :])
                    elif r0 >= PSLICE:
                        nc.sync.dma_start(p1[r0 - PSLICE:r1_ - PSLICE, :], pt[:nrow, :])
                    else:
                        k = PSLICE - r0
                        nc.sync.dma_start(p0[r0:PSLICE, :], pt[:k, :])
                        nc.sync.dma_start(p1[0:r1_ - PSLICE, :], pt[k:nrow, :])

            # ---------- AllGather p slices ----------
            nc.gpsimd.collective_compute(
                "AllGather", AluOp.bypass, replica_groups=[list(range(NCORES))],
                ins=[p0[:]], outs=[pg0[:]])
            nc.gpsimd.collective_compute(
                "AllGather", AluOp.bypass, replica_groups=[list(range(NCORES))],
                ins=[p1[:]], outs=[pg1[:]])

            # ---------- Layer 2 + output ----------
            call_i = 0
            cum16 = 0
            for c0 in range(0, NWIN, L2_CHUNK):
                cw = list(range(c0, min(c0 + L2_CHUNK, NWIN)))
                chunk_ranks = int(sum(ranks[1, wi, :].sum() for wi in cw))
                if chunk_ranks == 0:
                    continue
                msg = mpool.tile([P, chunk_ranks, P], dt.bfloat16, tag="msg2")
                base_rank = None
                while call_i < len(calls2):
                    bu, roff, nr = calls2[call_i]
                    if base_rank is None:
                        base_rank = roff
                    if roff - base_rank >= chunk_ranks:
                        break
                    n_idx = nr * P
                    tblap = pg0[:] if bu == 0 else pg1[:]
                    nc.gpsimd.dma_gather(
                        msg[:, roff - base_rank:roff - base_rank + nr, :],
                        tblap, i2_t[:, cum16:cum16 + n_idx // 16],
                        n_idx, n_idx, P, single_packet=False)
                    cum16 += n_idx // 16
                    call_i += 1
                for wi in cw:
                    segs = [(a - base_rank, b - base_rank) for a, b in wmap2[wi]]
                    pagg = ppool.tile([P, P], dt.float32, tag="pagg")
                    first = True
                    for a, b in segs:
                        for r in range(a, b):
                            S = spool.tile([P, P], dt.bfloat16, tag="S2")
                            nc.vector.tensor_tensor(
                                out=S[:], in0=d2_t[:, base_rank + r:base_rank + r + 1].to_broadcast([P, P]),
                                in1=iota_t[:], op=AluOp.is_equal)
                            nc.tensor.matmul(pagg[:], lhsT=S[:], rhs=msg[:, r, :],
                                             start=first, stop=(r == b - 1 and (a, b) == segs[-1]))
                            first = False
                    z = wpool.tile([P, OUT], dt.float32, tag="z")
                    nc.vector.tensor_tensor(out=z[:], in0=pagg[:, :OUT],
                                            in1=rc_t[:, wi:wi + 1].to_broadcast([P, OUT]),
                                            op=AluOp.mult)
                    nc.vector.tensor_tensor(out=z[:], in0=z[:], in1=qbuf[:, wi, :], op=AluOp.add)
                    nc.vector.tensor_tensor(out=z[:], in0=z[:], in1=b2_t[:], op=AluOp.add)
                    mneg = wpool.tile([P, 1], dt.float32, tag="mneg")
                    nc.vector.tensor_reduce(mneg[:], z[:], axis=mybir.AxisListType.X, op=AluOp.max, negate=True)
                    ez = wpool.tile([P, OUT], dt.float32, tag="ez")
                    nc.scalar.activation(ez[:], z[:], AF.Exp, bias=mneg[:])
                    ssum = wpool.tile([P, 1], dt.float32, tag="ssum")
                    nc.vector.tensor_reduce(ssum[:], ez[:], axis=mybir.AxisListType.X, op=AluOp.add)
                    lsum = wpool.tile([P, 1], dt.float32, tag="lsum")
                    nc.scalar.activation(lsum[:], ssum[:], AF.Ln)
                    nc.vector.tensor_tensor(out=lsum[:], in0=lsum[:], in1=mneg[:], op=AluOp.subtract)
                    ot = wpool.tile([P, OUT], dt.float32, tag="ot")
                    nc.vector.tensor_tensor(out=ot[:], in0=z[:], in1=lsum[:].to_broadcast([P, OUT]), op=AluOp.subtract)
                    nc.sync.dma_start(t_out[wi * P:(wi + 1) * P, :], ot[:])

    nc.compile()

    in_maps = []
    for c in range(NCORES):
        i1a, d1a, i2a, d2a = core_arrays[c]
        xoT = np.zeros((P, NPAD), bf16)
        xoT[:, :NLOC] = x[c * NLOC:(c + 1) * NLOC].T.astype(bf16)
        rcf = np.ones(NPAD, np.float32)
        rcf[:NLOC] = recip[c * NLOC:(c + 1) * NLOC]
        rcc = rcf.reshape(NWIN, P).T.copy()
        in_maps.append({
            "xlo": xlo, "xhi": xhi, "xoT": xoT,
            "i1": i1a, "d1": d1a, "i2": i2a, "d2": d2a,
            "w1l": np.asarray(w1_l).astype(bf16), "w1r": np.asarray(w1_r).astype(bf16),
            "w2l": np.ascontiguousarray(np.asarray(w2_l).astype(bf16).reshape(HID // P, P, OUT).transpose(1, 0, 2)), "w2r": np.ascontiguousarray(np.asarray(w2_r).astype(bf16).reshape(HID // P, P, OUT).transpose(1, 0, 2)),
            "b1": np.asarray(b1, np.float32).reshape(HID // P, P).T.copy(),
            "b2b": b2b_np, "rc": rcc,
            "iota": iota_np,
        })
    res = run_bass_kernel_spmd(nc, in_maps, list(range(NCORES)))
    out = np.concatenate([res.results[c]["out"][:NLOC] for c in range(NCORES)], axis=0)
    kernel.last_results = res
    kernel.last_nc = nc
    return out.astype(np.float32)


# revision 8
# speedup vs baseline: 1.0419x; 1.0419x over previous
"""GraphSAGE (2-layer, mean-agg) Trainium2 Bass kernel, 8-core SPMD.

Design: shard dst nodes across 8 cores (6250 each). Edges partitioned by dst
owner, sorted by dst, grouped into 128-dst windows. Messages fetched with
gpsimd dma_gather (bf16 tables, single_packet=False); segment-sum done on the
PE via per-rank selection-matrix matmuls accumulated in PSUM per window
(scatter-add CCE has a duplicate-index race on HW, so no scatters are used).
Layer-2 aggregates p = h @ w2_l (40->128-col padded bf16) instead of h
(512-dim): p shards are AllGathered in two 3125-row slices so gather indices
fit int16.
"""
import numpy as np
import ml_dtypes

N = 50000
E = 800000
DIN, HID, OUT = 128, 512, 40
NCORES = 8
NLOC = N // NCORES          # 6250
P = 128
NWIN = (NLOC + P - 1) // P  # 49
NPAD = NWIN * P             # 6272
XSPLIT = 32768              # x table split for int16 gather indices
PSLICE = NLOC // 2          # 3125: p-table allgather slice
L1_CHUNK = 2                # windows per L1 gather call group
L2_CHUNK = 4

bf16 = ml_dtypes.bfloat16


def _build_schedule(edge_index):
    """Per-core, per-layer edge orderings + the cross-core-common rank schedule."""
    src = np.asarray(edge_index[0], dtype=np.int64)
    dst = np.asarray(edge_index[1], dtype=np.int64)
    deg = np.bincount(dst, minlength=N).astype(np.float32)
    recip = 1.0 / np.maximum(deg, 1.0)

    per_core = []
    for c in range(NCORES):
        lo, hi = c * NLOC, (c + 1) * NLOC
        m = (dst >= lo) & (dst < hi)
        s, d = src[m], dst[m] - lo
        per_core.append((s, d))

    # bucket key per layer: L1 by src>=XSPLIT, L2 by (src % NLOC) // PSLICE
    def buckets(s):
        return [s >= XSPLIT, (s % NLOC) // PSLICE]

    # counts[layer][core][win][bucket]
    counts = np.zeros((2, NCORES, NWIN, 2), np.int64)
    percore_lists = []  # [core][layer][win][bucket] -> (gidx array, drel array)
    for c in range(NCORES):
        s, d = per_core[c]
        w = d // P
        bk = buckets(s)
        layers = []
        for L in range(2):
            b = bk[L].astype(np.int64)
            order = np.lexsort((b, w))  # by window, then bucket
            ss, dd, ww, bb = s[order], d[order], w[order], b[order]
            wins = []
            for wi in range(NWIN):
                sel = ww == wi
                ssw, ddw, bbw = ss[sel], dd[sel], bb[sel]
                ents = []
                for bu in range(2):
                    q = bbw == bu
                    sq, dq = ssw[q], ddw[q]
                    if L == 0:
                        gi = np.where(sq >= XSPLIT, sq - XSPLIT, sq)
                    else:
                        u = sq % NLOC
                        gi = (sq // NLOC) * PSLICE + (u % PSLICE)
                    counts[L, c, wi, bu] = len(sq)
                    ents.append((gi.astype(np.int64), (dq - wi * P).astype(np.int64)))
                wins.append(ents)
            layers.append(wins)
        percore_lists.append(layers)

    # common rank schedule: ranks[L][win][bucket] = ceil(max_c count /128), >=0
    ranks = np.zeros((2, NWIN, 2), np.int64)
    for L in range(2):
        mx = counts[L].max(axis=0)  # [NWIN, 2]
        ranks[L] = (mx + P - 1) // P
        for wi in range(NWIN):  # ensure every window has >=1 rank total
            if ranks[L, wi].sum() == 0:
                ranks[L, wi, 0] = 1
    return per_core, percore_lists, ranks, recip


def _wrap_call(flat_idx):
    """int16 wrapped layout for one gather call: slot i -> [i%16, i//16]."""
    n = len(flat_idx)
    w = flat_idx.astype(np.int16).reshape(n // 16, 16).T.copy()
    return np.tile(w, (8, 1))  # [128, n/16]


def _pack_layer(layers_for_core, ranks, L, chunk):
    """Build idx [128, T16] int16, drel [128, R] bf16 and call/window metadata.
    Call layout per chunk: [bucket0: win a..b segs][bucket1: win a..b segs]."""
    idx_cols, drel_cols = [], []
    calls = []      # (bucket, rank_off, n_ranks)
    win_ranges = [] # per window: list of (rank_start, rank_end)
    rank_off = 0
    for c0 in range(0, NWIN, chunk):
        cw = range(c0, min(c0 + chunk, NWIN))
        for bu in range(2):
            seg_ranks = int(sum(ranks[L, wi, bu] for wi in cw))
            if seg_ranks == 0:
                continue
            flat = np.zeros(seg_ranks * P, np.int64)
            drel = np.full(seg_ranks * P, -1, np.int64)
            off = 0
            for wi in cw:
                nr = int(ranks[L, wi, bu])
                if nr == 0:
                    continue
                gi, dq = layers_for_core[wi][bu]
                flat[off:off + len(gi)] = gi
                drel[off:off + len(dq)] = dq
                if len(win_ranges) <= wi - 0:
                    pass
                win_ranges.append((wi, rank_off + off // P, rank_off + off // P + nr))
                off += nr * P
            idx_cols.append(_wrap_call(flat))
            # drel slot i -> partition i%128, rank i//128
            drel_cols.append(drel.reshape(seg_ranks, P).T.astype(bf16))
            calls.append((bu, rank_off, seg_ranks))
            rank_off += seg_ranks
    idx_arr = np.concatenate(idx_cols, axis=1)
    drel_arr = np.concatenate(drel_cols, axis=1)
    # merge win_ranges into per-window lists
    wmap = [[] for _ in range(NWIN)]
    for wi, a, b in win_ranges:
        wmap[wi].append((a, b))
    return idx_arr, drel_arr, calls, wmap


def kernel(x, edge_index, w1_l, b1, w1_r, w2_l, b2, w2_r):
    import concourse.bacc as bacc
    import concourse.bass as bass
    import concourse.mybir as mybir
    import concourse.tile as tile
    from concourse.bass_utils import run_bass_kernel_spmd
    from concourse.library_config import mlp
    from concourse.masks import make_identity

    x = np.asarray(x, np.float32)
    per_core, percore_lists, ranks, recip = _build_schedule(np.asarray(edge_index))

    # ---- host-side packed arrays (same shapes on every core) ----
    core_arrays = []
    for c in range(NCORES):
        i1, d1, calls1, wmap1 = _pack_layer(percore_lists[c][0], ranks, 0, L1_CHUNK)
        i2, d2, calls2, wmap2 = _pack_layer(percore_lists[c][1], ranks, 1, L2_CHUNK)
        core_arrays.append((i1, d1, i2, d2))
    calls1, wmap1, calls2, wmap2 = calls1, wmap1, calls2, wmap2  # same all cores

    xlo = np.zeros((XSPLIT, DIN), bf16); xlo[:] = x[:XSPLIT].astype(bf16)
    xhi = np.zeros((N - XSPLIT, DIN), bf16); xhi[:] = x[XSPLIT:].astype(bf16)
    iota_np = np.tile(np.arange(P, dtype=np.float32), (P, 1)).astype(bf16)
    b2b_np = np.tile(np.asarray(b2, np.float32)[None, :], (P, 1))

    T16_1, R1 = core_arrays[0][0].shape[1], core_arrays[0][1].shape[1]
    T16_2, R2 = core_arrays[0][2].shape[1], core_arrays[0][3].shape[1]

    nc = bacc.Bacc("TRN2")
    dt = mybir.dt
    t_xlo = nc.declare_dram_parameter("xlo", [XSPLIT, DIN], dt.bfloat16, isOutput=False)
    t_xhi = nc.declare_dram_parameter("xhi", [N - XSPLIT, DIN], dt.bfloat16, isOutput=False)
    t_xoT = nc.declare_dram_parameter("xoT", [P, NPAD], dt.bfloat16, isOutput=False)
    t_i1 = nc.declare_dram_parameter("i1", [P, T16_1], dt.int16, isOutput=False)
    t_d1 = nc.declare_dram_parameter("d1", [P, R1], dt.bfloat16, isOutput=False)
    t_i2 = nc.declare_dram_parameter("i2", [P, T16_2], dt.int16, isOutput=False)
    t_d2 = nc.declare_dram_parameter("d2", [P, R2], dt.bfloat16, isOutput=False)
    t_w1l = nc.declare_dram_parameter("w1l", [DIN, HID], dt.bfloat16, isOutput=False)
    t_w1r = nc.declare_dram_parameter("w1r", [DIN, HID], dt.bfloat16, isOutput=False)
    t_w2l = nc.declare_dram_parameter("w2l", [P, HID // P, OUT], dt.bfloat16, isOutput=False)
    t_w2r = nc.declare_dram_parameter("w2r", [P, HID // P, OUT], dt.bfloat16, isOutput=False)
    t_b1 = nc.declare_dram_parameter("b1", [P, HID // P], dt.float32, isOutput=False)
    t_b2 = nc.declare_dram_parameter("b2b", [P, OUT], dt.float32, isOutput=False)
    t_rc = nc.declare_dram_parameter("rc", [P, NWIN], dt.float32, isOutput=False)
    t_iota = nc.declare_dram_parameter("iota", [P, P], dt.bfloat16, isOutput=False)
    t_iota4 = nc.declare_dram_parameter("iota4", [P, 4, P], dt.bfloat16, isOutput=False)
    t_out = nc.declare_dram_parameter("out", [NPAD, OUT], dt.float32, isOutput=True)

    p0 = nc.dram_tensor("p0", [PSLICE, P], dt.bfloat16)
    p1 = nc.dram_tensor("p1", [PSLICE, P], dt.bfloat16)
    pg0 = nc.dram_tensor("pg0", [NCORES * PSLICE, P], dt.bfloat16, addr_space="Shared")
    pg1 = nc.dram_tensor("pg1", [NCORES * PSLICE, P], dt.bfloat16, addr_space="Shared")

    AluOp = mybir.AluOpType
    AF = mybir.ActivationFunctionType

    with tile.TileContext(nc) as tc:
        with tc.tile_pool(name="const", bufs=1) as cpool, \
             tc.tile_pool(name="msg", bufs=2) as mpool, \
             tc.tile_pool(name="sm", bufs=3) as spool, \
             tc.tile_pool(name="work", bufs=3) as wpool, \
             tc.tile_pool(name="psumA", bufs=2, space="PSUM") as ppool, \
             tc.tile_pool(name="psumB", bufs=1, space="PSUM") as ppoolb:
            nc.gpsimd.load_library(mlp)
            ident = cpool.tile([P, P], dt.bfloat16)
            make_identity(nc, ident[:])
            iota_t = cpool.tile([P, P], dt.bfloat16)
            nc.sync.dma_start(iota_t[:], t_iota[:])
            iota4_t = cpool.tile([P, 4, P], dt.bfloat16)
            nc.sync.dma_start(iota4_t[:], t_iota4[:])
            i1_t = cpool.tile([P, T16_1], dt.int16)
            nc.sync.dma_start(i1_t[:], t_i1[:])
            d1_t = cpool.tile([P, R1], dt.bfloat16)
            nc.sync.dma_start(d1_t[:], t_d1[:])
            i2_t = cpool.tile([P, T16_2], dt.int16)
            nc.sync.dma_start(i2_t[:], t_i2[:])
            d2_t = cpool.tile([P, R2], dt.bfloat16)
            nc.sync.dma_start(d2_t[:], t_d2[:])
            xoT_t = cpool.tile([P, NPAD], dt.bfloat16)
            nc.sync.dma_start(xoT_t[:], t_xoT[:])
            w1l_t = cpool.tile([DIN, HID], dt.bfloat16)
            nc.sync.dma_start(w1l_t[:], t_w1l[:])
            w1r_t = cpool.tile([DIN, HID], dt.bfloat16)
            nc.sync.dma_start(w1r_t[:], t_w1r[:])
            w2l_t = cpool.tile([P, HID // P, OUT], dt.bfloat16)
            nc.sync.dma_start(w2l_t[:], t_w2l[:])
            w2r_t = cpool.tile([P, HID // P, OUT], dt.bfloat16)
            nc.sync.dma_start(w2r_t[:], t_w2r[:])
            b1_t = cpool.tile([P, HID // P], dt.float32)
            nc.sync.dma_start(b1_t[:], t_b1[:])
            b2_t = cpool.tile([P, OUT], dt.float32)
            nc.sync.dma_start(b2_t[:], t_b2[:])
            rc_t = cpool.tile([P, NWIN], dt.float32)
            nc.sync.dma_start(rc_t[:], t_rc[:])
            qbuf = cpool.tile([P, NWIN, OUT], dt.float32)

            # ---------- Layer 1 + stage B, chunked ----------
            call_i = 0
            cum16 = 0
            for c0 in range(0, NWIN, L1_CHUNK):
                cw = list(range(c0, min(c0 + L1_CHUNK, NWIN)))
                chunk_ranks = int(sum(ranks[0, wi, :].sum() for wi in cw))
                if chunk_ranks == 0:
                    continue
                msg = mpool.tile([P, chunk_ranks, DIN], dt.bfloat16, tag="msg1")
                base_rank = None
                # issue this chunk's gather calls
                local_off = 0
                while call_i < len(calls1):
                    bu, roff, nr = calls1[call_i]
                    # does this call belong to the current chunk?
                    if base_rank is None:
                        base_rank = roff
                    if roff - base_rank >= chunk_ranks:
                        break
                    n_idx = nr * P
                    tblap = t_xlo[:] if bu == 0 else t_xhi[:]
                    nc.gpsimd.dma_gather(
                        msg[:, roff - base_rank:roff - base_rank + nr, :],
                        tblap, i1_t[:, cum16:cum16 + n_idx // 16],
                        n_idx, n_idx, DIN, single_packet=False)
                    cum16 += n_idx // 16
                    local_off += nr
                    call_i += 1
                # per-window segmented reduction + stage B
                for wi in cw:
                    segs = [(a - base_rank, b - base_rank) for a, b in wmap1[wi]]
                    nseg = sum(b - a for a, b in segs)
                    pagg = ppool.tile([P, P], dt.float32, tag="pagg")
                    first = True
                    for a, b in segs:
                        r = a
                        while r < b:
                            kk = min(4, b - r)
                            S = spool.tile([P, 4, P], dt.bfloat16, tag="S1")
                            nc.vector.tensor_tensor(
                                out=S[:, :kk, :],
                                in0=d1_t[:, base_rank + r:base_rank + r + kk, None].to_broadcast([P, kk, P]),
                                in1=iota4_t[:, :kk, :], op=AluOp.is_equal)
                            for j in range(kk):
                                nc.tensor.matmul(pagg[:], lhsT=S[:, j, :], rhs=msg[:, r + j, :],
                                                 start=first, stop=(r + j == b - 1 and (a, b) == segs[-1]))
                                first = False
                            r += kk
                    am = wpool.tile([P, DIN], dt.bfloat16, tag="am")
                    nc.scalar.activation(am[:], pagg[:], AF.Copy, scale=rc_t[:, wi:wi + 1])
                    pamT = ppoolb.tile([P, P], dt.bfloat16, tag="pamT")
                    nc.tensor.transpose(out=pamT[:], in_=am[:], identity=ident[:])
                    amT = wpool.tile([P, P], dt.bfloat16, tag="amT")
                    nc.scalar.activation(amT[:], pamT[:], AF.Copy)
                    # h blocks + p/q
                    pq = ppool.tile([P, OUT], dt.float32, tag="pq")
                    qq = ppool.tile([P, OUT], dt.float32, tag="qq")
                    for bjj in range(HID // P):
                        ph = ppoolb.tile([P, P], dt.float32, tag="ph")
                        nc.tensor.matmul(ph[:], lhsT=w1l_t[:, bjj * P:(bjj + 1) * P], rhs=amT[:], start=True, stop=False)
                        nc.tensor.matmul(ph[:], lhsT=w1r_t[:, bjj * P:(bjj + 1) * P], rhs=xoT_t[:, wi * P:(wi + 1) * P], start=False, stop=True)
                        hT = wpool.tile([P, P], dt.bfloat16, tag="hT")
                        nc.scalar.activation(hT[:], ph[:], AF.Relu, bias=b1_t[:, bjj:bjj + 1])
                        nc.tensor.matmul(pq[:], lhsT=hT[:], rhs=w2l_t[:, bjj, :], start=(bjj == 0), stop=(bjj == 3))
                        nc.tensor.matmul(qq[:], lhsT=hT[:], rhs=w2r_t[:, bjj, :], start=(bjj == 0), stop=(bjj == 3))
                    nc.scalar.activation(qbuf[:, wi, :], qq[:], AF.Copy)
                    pt = wpool.tile([P, P], dt.bfloat16, tag="pt")
                    nc.vector.memset(pt[:], 0.0)
                    nc.scalar.activation(pt[:, :OUT], pq[:], AF.Copy)
                    r0, r1_ = wi * P, min((wi + 1) * P, NLOC)
                    nrow = r1_ - r0
                    if r1_ <= PSLICE:
                        nc.sync.dma_start(p0[r0:r1_, :], pt[:nrow, :])
                    elif r0 >= PSLICE:
                        nc.sync.dma_start(p1[r0 - PSLICE:r1_ - PSLICE, :], pt[:nrow, :])
                    else:
                        k = PSLICE - r0
                        nc.sync.dma_start(p0[r0:PSLICE, :], pt[:k, :])
                        nc.sync.dma_start(p1[0:r1_ - PSLICE, :], pt[k:nrow, :])

            # ---------- AllGather p slices ----------
            nc.gpsimd.collective_compute(
                "AllGather", AluOp.bypass, replica_groups=[list(range(NCORES))],
                ins=[p0[:]], outs=[pg0[:]])
            nc.gpsimd.collective_compute(
                "AllGather", AluOp.bypass, replica_groups=[list(range(NCORES))],
                ins=[p1[:]], outs=[pg1[:]])

            # ---------- Layer 2 + output ----------
            call_i = 0
            cum16 = 0
            for c0 in range(0, NWIN, L2_CHUNK):
                cw = list(range(c0, min(c0 + L2_CHUNK, NWIN)))
                chunk_ranks = int(sum(ranks[1, wi, :].sum() for wi in cw))
                if chunk_ranks == 0:
                    continue
                msg = mpool.tile([P, chunk_ranks, P], dt.bfloat16, tag="msg2")
                base_rank = None
                while call_i < len(calls2):
                    bu, roff, nr = calls2[call_i]
                    if base_rank is None:
                        base_rank = roff
                    if roff - base_rank >= chunk_ranks:
                        break
                    n_idx = nr * P
                    tblap = pg0[:] if bu == 0 else pg1[:]
                    nc.gpsimd.dma_gather(
                        msg[:, roff - base_rank:roff - base_rank + nr, :],
                        tblap, i2_t[:, cum16:cum16 + n_idx // 16],
                        n_idx, n_idx, P, single_packet=False)
                    cum16 += n_idx // 16
                    call_i += 1
                for wi in cw:
                    segs = [(a - base_rank, b - base_rank) for a, b in wmap2[wi]]
                    pagg = ppool.tile([P, P], dt.float32, tag="pagg")
                    first = True
                    for a, b in segs:
                        r = a
                        while r < b:
                            kk = min(4, b - r)
                            S = spool.tile([P, 4, P], dt.bfloat16, tag="S2")
                            nc.vector.tensor_tensor(
                                out=S[:, :kk, :],
                                in0=d2_t[:, base_rank + r:base_rank + r + kk, None].to_broadcast([P, kk, P]),
                                in1=iota4_t[:, :kk, :], op=AluOp.is_equal)
                            for j in range(kk):
                                nc.tensor.matmul(pagg[:], lhsT=S[:, j, :], rhs=msg[:, r + j, :],
                                                 start=first, stop=(r + j == b - 1 and (a, b) == segs[-1]))
                                first = False
                            r += kk
                    z = wpool.tile([P, OUT], dt.float32, tag="z")
                    nc.vector.tensor_tensor(out=z[:], in0=pagg[:, :OUT],
                                            in1=rc_t[:, wi:wi + 1].to_broadcast([P, OUT]),
                                            op=AluOp.mult)
                    nc.vector.tensor_tensor(out=z[:], in0=z[:], in1=qbuf[:, wi, :], op=AluOp.add)
                    nc.vector.tensor_tensor(out=z[:], in0=z[:], in1=b2_t[:], op=AluOp.add)
                    mneg = wpool.tile([P, 1], dt.float32, tag="mneg")
                    nc.vector.tensor_reduce(mneg[:], z[:], axis=mybir.AxisListType.X, op=AluOp.max, negate=True)
                    ez = wpool.tile([P, OUT], dt.float32, tag="ez")
                    nc.scalar.activation(ez[:], z[:], AF.Exp, bias=mneg[:])
                    ssum = wpool.tile([P, 1], dt.float32, tag="ssum")
                    nc.vector.tensor_reduce(ssum[:], ez[:], axis=mybir.AxisListType.X, op=AluOp.add)
                    lsum = wpool.tile([P, 1], dt.float32, tag="lsum")
                    nc.scalar.activation(lsum[:], ssum[:], AF.Ln)
                    nc.vector.tensor_tensor(out=lsum[:], in0=lsum[:], in1=mneg[:], op=AluOp.subtract)
                    ot = wpool.tile([P, OUT], dt.float32, tag="ot")
                    nc.vector.tensor_tensor(out=ot[:], in0=z[:], in1=lsum[:].to_broadcast([P, OUT]), op=AluOp.subtract)
                    nc.sync.dma_start(t_out[wi * P:(wi + 1) * P, :], ot[:])

    nc.compile()

    in_maps = []
    for c in range(NCORES):
        i1a, d1a, i2a, d2a = core_arrays[c]
        xoT = np.zeros((P, NPAD), bf16)
        xoT[:, :NLOC] = x[c * NLOC:(c + 1) * NLOC].T.astype(bf16)
        rcf = np.ones(NPAD, np.float32)
        rcf[:NLOC] = recip[c * NLOC:(c + 1) * NLOC]
        rcc = rcf.reshape(NWIN, P).T.copy()
        in_maps.append({
            "xlo": xlo, "xhi": xhi, "xoT": xoT,
            "i1": i1a, "d1": d1a, "i2": i2a, "d2": d2a,
            "w1l": np.asarray(w1_l).astype(bf16), "w1r": np.asarray(w1_r).astype(bf16),
            "w2l": np.ascontiguousarray(np.asarray(w2_l).astype(bf16).reshape(HID // P, P, OUT).transpose(1, 0, 2)), "w2r": np.ascontiguousarray(np.asarray(w2_r).astype(bf16).reshape(HID // P, P, OUT).transpose(1, 0, 2)),
            "b1": np.asarray(b1, np.float32).reshape(HID // P, P).T.copy(),
            "b2b": b2b_np, "rc": rcc,
            "iota": iota_np, "iota4": np.ascontiguousarray(np.broadcast_to(iota_np[:, None, :], (128, 4, 128))),
        })
    res = run_bass_kernel_spmd(nc, in_maps, list(range(NCORES)))
    out = np.concatenate([res.results[c]["out"][:NLOC] for c in range(NCORES)], axis=0)
    kernel.last_results = res
    kernel.last_nc = nc
    return out.astype(np.float32)


# revision 9
# speedup vs baseline: 1.0715x; 1.0284x over previous
"""GraphSAGE (2-layer, mean-agg) Trainium2 Bass kernel, 8-core SPMD.

Design: shard dst nodes across 8 cores (6250 each). Edges partitioned by dst
owner, sorted by dst, grouped into 128-dst windows. Messages fetched with
gpsimd dma_gather (bf16 tables, single_packet=False); segment-sum done on the
PE via per-rank selection-matrix matmuls accumulated in PSUM per window
(scatter-add CCE has a duplicate-index race on HW, so no scatters are used).
Layer-2 aggregates p = h @ w2_l (40->128-col padded bf16) instead of h
(512-dim): p shards are AllGathered in two 3125-row slices so gather indices
fit int16.
"""
import numpy as np
import ml_dtypes

N = 50000
E = 800000
DIN, HID, OUT = 128, 512, 40
NCORES = 8
NLOC = N // NCORES          # 6250
P = 128
NWIN = (NLOC + P - 1) // P  # 49
NPAD = NWIN * P             # 6272
XSPLIT = 32768              # x table split for int16 gather indices
SLICE_LEN = 1568            # p-table allgather slice length (4 slices)
NSLICE = 4
L1_CHUNK = 2                # windows per L1 gather call group
L2_CHUNK = 4

bf16 = ml_dtypes.bfloat16


def _build_schedule(edge_index):
    """Per-core, per-layer edge orderings + the cross-core-common rank schedule."""
    src = np.asarray(edge_index[0], dtype=np.int64)
    dst = np.asarray(edge_index[1], dtype=np.int64)
    deg = np.bincount(dst, minlength=N).astype(np.float32)
    recip = 1.0 / np.maximum(deg, 1.0)

    per_core = []
    for c in range(NCORES):
        lo, hi = c * NLOC, (c + 1) * NLOC
        m = (dst >= lo) & (dst < hi)
        s, d = src[m], dst[m] - lo
        per_core.append((s, d))

    # bucket key per layer: L1 by src>=XSPLIT, L2 by (src % NLOC) // PSLICE
    def buckets(s):
        return [s >= XSPLIT, (s % NLOC) // SLICE_LEN]

    # counts[layer][core][win][bucket]
    counts = np.zeros((2, NCORES, NWIN, 4), np.int64)
    percore_lists = []  # [core][layer][win][bucket] -> (gidx array, drel array)
    for c in range(NCORES):
        s, d = per_core[c]
        w = d // P
        bk = buckets(s)
        layers = []
        for L in range(2):
            nb = 2 if L == 0 else NSLICE
            b = bk[L].astype(np.int64)
            order = np.lexsort((b, w))  # by window, then bucket
            ss, dd, ww, bb = s[order], d[order], w[order], b[order]
            wins = []
            for wi in range(NWIN):
                sel = ww == wi
                ssw, ddw, bbw = ss[sel], dd[sel], bb[sel]
                ents = []
                for bu in range(nb):
                    q = bbw == bu
                    sq, dq = ssw[q], ddw[q]
                    if L == 0:
                        gi = np.where(sq >= XSPLIT, sq - XSPLIT, sq)
                    else:
                        u = sq % NLOC
                        gi = (sq // NLOC) * SLICE_LEN + (u - (u // SLICE_LEN) * SLICE_LEN)
                    counts[L, c, wi, bu] = len(sq)
                    ents.append((gi.astype(np.int64), (dq - wi * P).astype(np.int64)))
                wins.append(ents)
            layers.append(wins)
        percore_lists.append(layers)

    # common rank schedule: ranks[L][win][bucket] = ceil(max_c count /128), >=0
    ranks = np.zeros((2, NWIN, 4), np.int64)
    for L in range(2):
        mx = counts[L].max(axis=0)  # [NWIN, 2]
        ranks[L] = (mx + P - 1) // P
        for wi in range(NWIN):  # ensure every window has >=1 rank total
            if ranks[L, wi].sum() == 0:
                ranks[L, wi, 0] = 1
    return per_core, percore_lists, ranks, recip


def _wrap_call(flat_idx):
    """int16 wrapped layout for one gather call: slot i -> [i%16, i//16]."""
    n = len(flat_idx)
    w = flat_idx.astype(np.int16).reshape(n // 16, 16).T.copy()
    return np.tile(w, (8, 1))  # [128, n/16]


def _pack_layer(layers_for_core, ranks, L, chunk):
    """Build idx [128, T16] int16, drel [128, R] bf16 and call/window metadata.
    Call layout per chunk: [bucket0: win a..b segs][bucket1: win a..b segs]."""
    idx_cols, drel_cols = [], []
    calls = []      # (bucket, rank_off, n_ranks)
    win_ranges = [] # per window: list of (rank_start, rank_end)
    rank_off = 0
    for c0 in range(0, NWIN, chunk):
        cw = range(c0, min(c0 + chunk, NWIN))
        for bu in range(2 if L == 0 else NSLICE):
            seg_ranks = int(sum(ranks[L, wi, bu] for wi in cw))
            if seg_ranks == 0:
                continue
            flat = np.zeros(seg_ranks * P, np.int64)
            drel = np.full(seg_ranks * P, -1, np.int64)
            off = 0
            for wi in cw:
                nr = int(ranks[L, wi, bu])
                if nr == 0:
                    continue
                gi, dq = layers_for_core[wi][bu]
                flat[off:off + len(gi)] = gi
                drel[off:off + len(dq)] = dq
                if len(win_ranges) <= wi - 0:
                    pass
                win_ranges.append((wi, rank_off + off // P, rank_off + off // P + nr))
                off += nr * P
            idx_cols.append(_wrap_call(flat))
            # drel slot i -> partition i%128, rank i//128
            drel_cols.append(drel.reshape(seg_ranks, P).T.astype(bf16))
            calls.append((bu, rank_off, seg_ranks))
            rank_off += seg_ranks
    idx_arr = np.concatenate(idx_cols, axis=1)
    drel_arr = np.concatenate(drel_cols, axis=1)
    # merge win_ranges into per-window lists
    wmap = [[] for _ in range(NWIN)]
    for wi, a, b in win_ranges:
        wmap[wi].append((a, b))
    return idx_arr, drel_arr, calls, wmap


def kernel(x, edge_index, w1_l, b1, w1_r, w2_l, b2, w2_r):
    import concourse.bacc as bacc
    import concourse.bass as bass
    import concourse.mybir as mybir
    import concourse.tile as tile
    from concourse.bass_utils import run_bass_kernel_spmd
    from concourse.library_config import mlp
    from concourse.masks import make_identity

    x = np.asarray(x, np.float32)
    per_core, percore_lists, ranks, recip = _build_schedule(np.asarray(edge_index))

    # ---- host-side packed arrays (same shapes on every core) ----
    core_arrays = []
    for c in range(NCORES):
        i1, d1, calls1, wmap1 = _pack_layer(percore_lists[c][0], ranks, 0, L1_CHUNK)
        i2, d2, calls2, wmap2 = _pack_layer(percore_lists[c][1], ranks, 1, L2_CHUNK)
        core_arrays.append((i1, d1, i2, d2))
    calls1, wmap1, calls2, wmap2 = calls1, wmap1, calls2, wmap2  # same all cores

    xlo = np.zeros((XSPLIT, DIN), bf16); xlo[:] = x[:XSPLIT].astype(bf16)
    xhi = np.zeros((N - XSPLIT, DIN), bf16); xhi[:] = x[XSPLIT:].astype(bf16)
    iota_np = np.tile(np.arange(P, dtype=np.float32), (P, 1)).astype(bf16)
    b2b_np = np.tile(np.asarray(b2, np.float32)[None, :], (P, 1))

    T16_1, R1 = core_arrays[0][0].shape[1], core_arrays[0][1].shape[1]
    T16_2, R2 = core_arrays[0][2].shape[1], core_arrays[0][3].shape[1]

    nc = bacc.Bacc("TRN2")
    dt = mybir.dt
    t_xlo = nc.declare_dram_parameter("xlo", [XSPLIT, DIN], dt.bfloat16, isOutput=False)
    t_xhi = nc.declare_dram_parameter("xhi", [N - XSPLIT, DIN], dt.bfloat16, isOutput=False)
    t_xoT = nc.declare_dram_parameter("xoT", [P, NPAD], dt.bfloat16, isOutput=False)
    t_i1 = nc.declare_dram_parameter("i1", [P, T16_1], dt.int16, isOutput=False)
    t_d1 = nc.declare_dram_parameter("d1", [P, R1], dt.bfloat16, isOutput=False)
    t_i2 = nc.declare_dram_parameter("i2", [P, T16_2], dt.int16, isOutput=False)
    t_d2 = nc.declare_dram_parameter("d2", [P, R2], dt.bfloat16, isOutput=False)
    t_w1l = nc.declare_dram_parameter("w1l", [DIN, HID], dt.bfloat16, isOutput=False)
    t_w1r = nc.declare_dram_parameter("w1r", [DIN, HID], dt.bfloat16, isOutput=False)
    t_w2l = nc.declare_dram_parameter("w2l", [P, HID // P, OUT], dt.bfloat16, isOutput=False)
    t_w2r = nc.declare_dram_parameter("w2r", [P, HID // P, OUT], dt.bfloat16, isOutput=False)
    t_b1 = nc.declare_dram_parameter("b1", [P, HID // P], dt.float32, isOutput=False)
    t_b2 = nc.declare_dram_parameter("b2b", [P, OUT], dt.float32, isOutput=False)
    t_rc = nc.declare_dram_parameter("rc", [P, NWIN], dt.float32, isOutput=False)
    t_iota = nc.declare_dram_parameter("iota", [P, P], dt.bfloat16, isOutput=False)
    t_iota4 = nc.declare_dram_parameter("iota4", [P, 4, P], dt.bfloat16, isOutput=False)
    t_out = nc.declare_dram_parameter("out", [NPAD, OUT], dt.float32, isOutput=True)

    pS = [nc.dram_tensor(f"p{s}", [SLICE_LEN, P], dt.bfloat16) for s in range(NSLICE)]
    pgS = [nc.dram_tensor(f"pg{s}", [NCORES * SLICE_LEN, P], dt.bfloat16, addr_space="Shared") for s in range(NSLICE)]

    AluOp = mybir.AluOpType
    AF = mybir.ActivationFunctionType

    with tile.TileContext(nc) as tc:
        with tc.tile_pool(name="const", bufs=1) as cpool, \
             tc.tile_pool(name="msg", bufs=2) as mpool, \
             tc.tile_pool(name="sm", bufs=3) as spool, \
             tc.tile_pool(name="work", bufs=3) as wpool, \
             tc.tile_pool(name="psumA", bufs=2, space="PSUM") as ppool, \
             tc.tile_pool(name="psumB", bufs=1, space="PSUM") as ppoolb:
            nc.gpsimd.load_library(mlp)
            ident = cpool.tile([P, P], dt.bfloat16)
            make_identity(nc, ident[:])
            iota_t = cpool.tile([P, P], dt.bfloat16)
            nc.sync.dma_start(iota_t[:], t_iota[:])
            iota4_t = cpool.tile([P, 4, P], dt.bfloat16)
            nc.sync.dma_start(iota4_t[:], t_iota4[:])
            i1_t = cpool.tile([P, T16_1], dt.int16)
            nc.sync.dma_start(i1_t[:], t_i1[:])
            d1_t = cpool.tile([P, R1], dt.bfloat16)
            nc.sync.dma_start(d1_t[:], t_d1[:])
            i2_t = cpool.tile([P, T16_2], dt.int16)
            nc.sync.dma_start(i2_t[:], t_i2[:])
            d2_t = cpool.tile([P, R2], dt.bfloat16)
            nc.sync.dma_start(d2_t[:], t_d2[:])
            xoT_t = cpool.tile([P, NPAD], dt.bfloat16)
            nc.sync.dma_start(xoT_t[:], t_xoT[:])
            w1l_t = cpool.tile([DIN, HID], dt.bfloat16)
            nc.sync.dma_start(w1l_t[:], t_w1l[:])
            w1r_t = cpool.tile([DIN, HID], dt.bfloat16)
            nc.sync.dma_start(w1r_t[:], t_w1r[:])
            w2l_t = cpool.tile([P, HID // P, OUT], dt.bfloat16)
            nc.sync.dma_start(w2l_t[:], t_w2l[:])
            w2r_t = cpool.tile([P, HID // P, OUT], dt.bfloat16)
            nc.sync.dma_start(w2r_t[:], t_w2r[:])
            b1_t = cpool.tile([P, HID // P], dt.float32)
            nc.sync.dma_start(b1_t[:], t_b1[:])
            b2_t = cpool.tile([P, OUT], dt.float32)
            nc.sync.dma_start(b2_t[:], t_b2[:])
            rc_t = cpool.tile([P, NWIN], dt.float32)
            nc.sync.dma_start(rc_t[:], t_rc[:])
            qbuf = cpool.tile([P, NWIN, OUT], dt.float32)

            # ---------- Layer 1 + stage B, chunked ----------
            call_i = 0
            cum16 = 0
            for c0 in range(0, NWIN, L1_CHUNK):
                cw = list(range(c0, min(c0 + L1_CHUNK, NWIN)))
                chunk_ranks = int(sum(ranks[0, wi, :].sum() for wi in cw))
                if chunk_ranks == 0:
                    continue
                msg = mpool.tile([P, chunk_ranks, DIN], dt.bfloat16, tag="msg1")
                base_rank = None
                # issue this chunk's gather calls
                local_off = 0
                while call_i < len(calls1):
                    bu, roff, nr = calls1[call_i]
                    # does this call belong to the current chunk?
                    if base_rank is None:
                        base_rank = roff
                    if roff - base_rank >= chunk_ranks:
                        break
                    n_idx = nr * P
                    tblap = t_xlo[:] if bu == 0 else t_xhi[:]
                    nc.gpsimd.dma_gather(
                        msg[:, roff - base_rank:roff - base_rank + nr, :],
                        tblap, i1_t[:, cum16:cum16 + n_idx // 16],
                        n_idx, n_idx, DIN, single_packet=False)
                    cum16 += n_idx // 16
                    local_off += nr
                    call_i += 1
                # per-window segmented reduction + stage B
                for wi in cw:
                    segs = [(a - base_rank, b - base_rank) for a, b in wmap1[wi]]
                    nseg = sum(b - a for a, b in segs)
                    pagg = ppool.tile([P, P], dt.float32, tag="pagg")
                    first = True
                    for a, b in segs:
                        r = a
                        while r < b:
                            kk = min(4, b - r)
                            S = spool.tile([P, 4, P], dt.bfloat16, tag="S1")
                            nc.vector.tensor_tensor(
                                out=S[:, :kk, :],
                                in0=d1_t[:, base_rank + r:base_rank + r + kk, None].to_broadcast([P, kk, P]),
                                in1=iota4_t[:, :kk, :], op=AluOp.is_equal)
                            for j in range(kk):
                                nc.tensor.matmul(pagg[:], lhsT=S[:, j, :], rhs=msg[:, r + j, :],
                                                 start=first, stop=(r + j == b - 1 and (a, b) == segs[-1]))
                                first = False
                            r += kk
                    am = wpool.tile([P, DIN], dt.bfloat16, tag="am")
                    nc.scalar.activation(am[:], pagg[:], AF.Copy, scale=rc_t[:, wi:wi + 1])
                    pamT = ppoolb.tile([P, P], dt.bfloat16, tag="pamT")
                    nc.tensor.transpose(out=pamT[:], in_=am[:], identity=ident[:])
                    amT = wpool.tile([P, P], dt.bfloat16, tag="amT")
                    nc.scalar.activation(amT[:], pamT[:], AF.Copy)
                    # h blocks + p/q
                    pq = ppool.tile([P, OUT], dt.float32, tag="pq")
                    qq = ppool.tile([P, OUT], dt.float32, tag="qq")
                    for bjj in range(HID // P):
                        ph = ppoolb.tile([P, P], dt.float32, tag="ph")
                        nc.tensor.matmul(ph[:], lhsT=w1l_t[:, bjj * P:(bjj + 1) * P], rhs=amT[:], start=True, stop=False)
                        nc.tensor.matmul(ph[:], lhsT=w1r_t[:, bjj * P:(bjj + 1) * P], rhs=xoT_t[:, wi * P:(wi + 1) * P], start=False, stop=True)
                        hT = wpool.tile([P, P], dt.bfloat16, tag="hT")
                        nc.scalar.activation(hT[:], ph[:], AF.Relu, bias=b1_t[:, bjj:bjj + 1])
                        nc.tensor.matmul(pq[:], lhsT=hT[:], rhs=w2l_t[:, bjj, :], start=(bjj == 0), stop=(bjj == 3))
                        nc.tensor.matmul(qq[:], lhsT=hT[:], rhs=w2r_t[:, bjj, :], start=(bjj == 0), stop=(bjj == 3))
                    nc.scalar.activation(qbuf[:, wi, :], qq[:], AF.Copy)
                    pt = wpool.tile([P, P], dt.bfloat16, tag="pt")
                    nc.vector.memset(pt[:], 0.0)
                    nc.scalar.activation(pt[:, :OUT], pq[:], AF.Copy)
                    r0, r1_ = wi * P, min((wi + 1) * P, NLOC)
                    for s in range(NSLICE):
                        a0, a1 = s * SLICE_LEN, min((s + 1) * SLICE_LEN, NLOC)
                        c0_, c1_ = max(r0, a0), min(r1_, a1)
                        if c0_ < c1_:
                            nc.sync.dma_start(pS[s][c0_ - a0:c1_ - a0, :], pt[c0_ - r0:c1_ - r0, :])

            # ---------- AllGather p slices ----------
            for s in range(NSLICE):
                nc.gpsimd.collective_compute(
                    "AllGather", AluOp.bypass, replica_groups=[list(range(NCORES))],
                    ins=[pS[s][:]], outs=[pgS[s][:]])

            # ---------- Layer 2 + output ----------
            call_i = 0
            cum16 = 0
            for c0 in range(0, NWIN, L2_CHUNK):
                cw = list(range(c0, min(c0 + L2_CHUNK, NWIN)))
                chunk_ranks = int(sum(ranks[1, wi, :].sum() for wi in cw))
                if chunk_ranks == 0:
                    continue
                msg = mpool.tile([P, chunk_ranks, P], dt.bfloat16, tag="msg2")
                base_rank = None
                while call_i < len(calls2):
                    bu, roff, nr = calls2[call_i]
                    if base_rank is None:
                        base_rank = roff
                    if roff - base_rank >= chunk_ranks:
                        break
                    n_idx = nr * P
                    tblap = pgS[bu][:]
                    nc.gpsimd.dma_gather(
                        msg[:, roff - base_rank:roff - base_rank + nr, :],
                        tblap, i2_t[:, cum16:cum16 + n_idx // 16],
                        n_idx, n_idx, P, single_packet=False)
                    cum16 += n_idx // 16
                    call_i += 1
                for wi in cw:
                    segs = [(a - base_rank, b - base_rank) for a, b in wmap2[wi]]
                    pagg = ppool.tile([P, P], dt.float32, tag="pagg")
                    first = True
                    for a, b in segs:
                        r = a
                        while r < b:
                            kk = min(4, b - r)
                            S = spool.tile([P, 4, P], dt.bfloat16, tag="S2")
                            nc.vector.tensor_tensor(
                                out=S[:, :kk, :],
                                in0=d2_t[:, base_rank + r:base_rank + r + kk, None].to_broadcast([P, kk, P]),
                                in1=iota4_t[:, :kk, :], op=AluOp.is_equal)
                            for j in range(kk):
                                nc.tensor.matmul(pagg[:], lhsT=S[:, j, :], rhs=msg[:, r + j, :],
                                                 start=first, stop=(r + j == b - 1 and (a, b) == segs[-1]))
                                first = False
                            r += kk
                    z = wpool.tile([P, OUT], dt.float32, tag="z")
                    nc.vector.tensor_tensor(out=z[:], in0=pagg[:, :OUT],
                                            in1=rc_t[:, wi:wi + 1].to_broadcast([P, OUT]),
                                            op=AluOp.mult)
                    nc.vector.tensor_tensor(out=z[:], in0=z[:], in1=qbuf[:, wi, :], op=AluOp.add)
                    nc.vector.tensor_tensor(out=z[:], in0=z[:], in1=b2_t[:], op=AluOp.add)
                    mneg = wpool.tile([P, 1], dt.float32, tag="mneg")
                    nc.vector.tensor_reduce(mneg[:], z[:], axis=mybir.AxisListType.X, op=AluOp.max, negate=True)
                    ez = wpool.tile([P, OUT], dt.float32, tag="ez")
                    nc.scalar.activation(ez[:], z[:], AF.Exp, bias=mneg[:])
                    ssum = wpool.tile([P, 1], dt.float32, tag="ssum")
                    nc.vector.tensor_reduce(ssum[:], ez[:], axis=mybir.AxisListType.X, op=AluOp.add)
                    lsum = wpool.tile([P, 1], dt.float32, tag="lsum")
                    nc.scalar.activation(lsum[:], ssum[:], AF.Ln)
                    nc.vector.tensor_tensor(out=lsum[:], in0=lsum[:], in1=mneg[:], op=AluOp.subtract)
                    ot = wpool.tile([P, OUT], dt.float32, tag="ot")
                    nc.vector.tensor_tensor(out=ot[:], in0=z[:], in1=lsum[:].to_broadcast([P, OUT]), op=AluOp.subtract)
                    nc.sync.dma_start(t_out[wi * P:(wi + 1) * P, :], ot[:])

    nc.compile()

    in_maps = []
    for c in range(NCORES):
        i1a, d1a, i2a, d2a = core_arrays[c]
        xoT = np.zeros((P, NPAD), bf16)
        xoT[:, :NLOC] = x[c * NLOC:(c + 1) * NLOC].T.astype(bf16)
        rcf = np.ones(NPAD, np.float32)
        rcf[:NLOC] = recip[c * NLOC:(c + 1) * NLOC]
        rcc = rcf.reshape(NWIN, P).T.copy()
        in_maps.append({
            "xlo": xlo, "xhi": xhi, "xoT": xoT,
            "i1": i1a, "d1": d1a, "i2": i2a, "d2": d2a,
            "w1l": np.asarray(w1_l).astype(bf16), "w1r": np.asarray(w1_r).astype(bf16),
            "w2l": np.ascontiguousarray(np.asarray(w2_l).astype(bf16).reshape(HID // P, P, OUT).transpose(1, 0, 2)), "w2r": np.ascontiguousarray(np.asarray(w2_r).astype(bf16).reshape(HID // P, P, OUT).transpose(1, 0, 2)),
            "b1": np.asarray(b1, np.float32).reshape(HID // P, P).T.copy(),
            "b2b": b2b_np, "rc": rcc,
            "iota": iota_np, "iota4": np.ascontiguousarray(np.broadcast_to(iota_np[:, None, :], (128, 4, 128))),
        })
    res = run_bass_kernel_spmd(nc, in_maps, list(range(NCORES)))
    out = np.concatenate([res.results[c]["out"][:NLOC] for c in range(NCORES)], axis=0)
    kernel.last_results = res
    kernel.last_nc = nc
    return out.astype(np.float32)
